# revision 1
# baseline (speedup 1.0000x reference)
"""Trainium2 Bass kernel for nn_Diagnet (S=1024, B=64, I=512, H=2048, O=512).

    u = einsum('sbi,hi->sbh', X, W_ih)
    h_t = |u_t + hh * h_{t-1}|   (scan over S, only final h needed)
    Y = h_final @ W_ho.T + b_ho

Strategy (8 NeuronCores, data-parallel over batch, BC=8 rows per core):

* H lanes are permuted so hh is sorted descending and split into 16
  chunks of 128.  A chunk whose largest decay a satisfies a^K < tol
  only needs the last K steps (exact to ~tol relative), so each chunk
  gets a window K_g (multiple of 32), and the GEMM + scan skip
  everything earlier.
* The recurrence is computed by a custom DVE instruction that folds
  a whole window in one go: out[tau] = |out[tau-1] - u[tau]*scn[tau]|
  via scan(ABSOLUTE_DIFF, Src0*Src1).  The running state lives in the
  engine (no SBUF round-trip per step), so the serial chain that
  dominated the naive per-step implementation (~200ns x 1024 steps)
  collapses to ~K-cycle streaming instructions.  scn[tau] =
  -a_lane^(K-1-tau) folds the per-step decay multiply into a prescale
  (a>=0 lets a*|x| = |a x|), and the minus sign turns ABSOLUTE_DIFF
  into abs-add.  h_final = last scan element (scale 1).
* For chunks 1..15 all 8 batch rows fold in ONE instruction: a BIG
  separator pair between rows, scaled by (-1, +1), absorbs and then
  exactly zeroes the running state (|m+BIG| rounds to BIG in fp32,
  then |BIG-BIG| = 0).  Chunk 0 (full 1024-step window) is scanned in
  ascending segments that chase the X DMA stream, each segment seeded
  per batch row from the previous segment's last element (init=C0).
* GEMM runs in bf16 (1 cycle/row on the PE vs 4 for fp32; X DMA
  halves).  X is host-tiled to [block, 128i, (ic,b,tau)] and kept
  resident in SBUF; chunks are processed newest-window-first, so the
  13 single-block chunks run off the first-arriving X blocks while
  chunk 0's ascending blocks stream in; i-chunk-outer PSUM runs
  amortize LDWEIGHTS, and dependency-free warm-up matmuls lift the
  PE HAM clock gate before real work lands.
* The Activation engine drains PSUM -> SBUF with a pure layout copy
  (to b-major contiguous windows); DVE extracts h_final columns (cast
  to bf16); the final projection is 16 accumulating bf16 matmuls
  (emitted last so they never stall the PE FIFO) + bias add.
"""

import math
import os

from contextlib import ExitStack

import numpy as np

S, B, I, H, O = 1024, 64, 512, 2048, 512
NCORES = 8
BC = B // NCORES  # 8 batch rows per core
TB = 64  # X block granularity
WG = 32  # truncation-window granularity (half-block)
NBLK = S // TB  # 16
NCH = H // 128  # 16 h-chunks
NI = I // 128  # 4 i-chunks
USMALL_W = 256  # max window (cols) for chunks g>=1; K_1 <= 256 needs LN <= ~16

_CACHE = {}


def _register_scan_ops():
    """Two fold ops: m[t] = |m[t-1] - in0[t]*in1[t]|, seeded with 0 or with a
    per-partition value (s0) for chaining segment scans."""
    import concourse.dve_ops as dve_ops
    from concourse.dve_spec import C0, Spec, Src0, Src1, Zero, scan, lower, AluOp
    from concourse.dve_uop import DveOpSpec

    have = {op.name: op for op in dve_ops.OPS}
    if "ABSDIFF_SCALE_SCAN_ANT" in have:
        return have["ABSDIFF_SCALE_SCAN_ANT"], have["ABSDIFF_SCALE_SCAN_SEED_ANT"]

    def _ref_factory(seeded):
        def _ref(in0, in1, s0, s1, imm2):
            x = in0.astype(np.float32) * in1.astype(np.float32)
            out = np.empty_like(x)
            m = (
                np.broadcast_to(np.asarray(s0, np.float32).reshape(-1), (x.shape[0],))
                if seeded
                else np.zeros(x.shape[0], np.float32)
            ).copy()
            for t in range(x.shape[1]):
                m = np.abs(m - x[:, t])
                out[:, t] = m
            return out

        return _ref

    ops = []
    for name, init, seeded in (
        ("ABSDIFF_SCALE_SCAN_ANT", Zero, False),
        ("ABSDIFF_SCALE_SCAN_SEED_ANT", C0, True),
    ):
        spec = Spec(
            body=scan(AluOp.ABSOLUTE_DIFF, Src0 * Src1, init=init),
            reference=_ref_factory(seeded),
        )
        row = max(dve_ops._SUB_OPCODE_FOR_NAME.values()) + 1
        assert row < 0x20
        shas = {}
        for ver in ("v3", "v4"):
            s = DveOpSpec(name=name, opcode=row, uops=lower(spec, ver=ver), rd1_en=True)
            shas[ver] = s.sha(ver)
        op = dve_ops.DveOp(name, spec, subdim=False, uops_sha=shas)
        dve_ops._SUB_OPCODE_FOR_NAME[name] = row
        dve_ops.OPS.append(op)
        dve_ops.CUSTOM_DVE_SPECS[name] = spec
        ops.append(op)
    return ops[0], ops[1]


def _make_plan(hh):
    ln = float(os.environ.get("DIAG_LN", "7.0"))  # a^K <= e^-ln truncation tol
    a = np.maximum(np.abs(hh.astype(np.float64)), 1e-30)
    perm = np.argsort(-a, kind="stable")
    ag = a[perm].reshape(NCH, 128)  # [chunk, lane], descending
    windows = []
    for g in range(NCH):
        amax = ag[g, 0]
        if S * math.log(amax) >= -ln:
            kg = S
        else:
            kg = int(math.ceil(ln / math.log(1.0 / amax)))
        kg = min(S, max(WG, ((kg + WG - 1) // WG) * WG))
        windows.append(kg)
    assert all(windows[g] >= windows[g + 1] for g in range(NCH - 1)), windows
    assert all(k <= USMALL_W for k in windows[1:]), (windows, "raise USMALL_W")
    # chunk 0: K cols.  chunks g>=1: K+2 cols, the extra two being the
    # batch-separator scales (-1, +1) for the fused multi-batch scan.
    widths = [windows[0]] + [k + 2 for k in windows[1:]]
    offs = np.concatenate([[0], np.cumsum(widths)]).astype(int)
    scn = np.zeros((128, offs[-1]), dtype=np.float64)
    for g in range(NCH):
        kg = windows[g]
        tau = np.arange(kg)
        scn[:, offs[g] : offs[g] + kg] = -(ag[g][:, None] ** (kg - 1 - tau)[None, :])
        if g >= 1:
            scn[:, offs[g] + kg] = -1.0
            scn[:, offs[g] + kg + 1] = 1.0
    return {
        "perm": perm,
        "windows": tuple(windows),
        "offs": offs,
        "SCN": scn,  # float64; cast at the call site
    }


def _build(windows, offs_total):
    import concourse.mybir as mybir
    import concourse.tile as tile
    from concourse import bacc
    from concourse.bass import ds

    SCAN_OP, SCAN_SEED_OP = _register_scan_ops()
    f32 = mybir.dt.float32
    bf16 = mybir.dt.bfloat16
    R = int(os.environ.get("DIAG_R", "6"))

    nc = bacc.Bacc("TRN2", target_bir_lowering=False, debug=False, num_devices=NCORES)
    # X block layout: partition p (= i within chunk), line [ic, b, tau] (4KB bf16)
    X = nc.dram_tensor("X", [NBLK, 128, NI * TB * BC], bf16, kind="ExternalInput").ap()
    # WIHT line: [g, ic, hsub] (per-chunk contiguous pieces); WHOT line: [g, o]
    WIHT = nc.dram_tensor("WIHT", [128, NCH * NI * 128], bf16, kind="ExternalInput").ap()
    WHOT = nc.dram_tensor("WHOT", [128, NCH * O], bf16, kind="ExternalInput").ap()
    SCN = nc.dram_tensor("SCN", [128, offs_total], bf16, kind="ExternalInput").ap()
    BIAS = nc.dram_tensor("BIAS", [BC, O], f32, kind="ExternalInput").ap()
    Y = nc.dram_tensor("Y", [BC, O], f32, kind="ExternalOutput").ap()

    widths = [windows[0]] + [k + 2 for k in windows[1:]]
    offs = np.concatenate([[0], np.cumsum(widths)]).astype(int)

    with tile.TileContext(nc) as tc:
        with ExitStack() as ctx:
            consts = ctx.enter_context(tc.tile_pool(name="consts", bufs=1))
            xpool = ctx.enter_context(tc.tile_pool(name="xt", bufs=1))
            ubig = ctx.enter_context(tc.tile_pool(name="ubig", bufs=1))
            usmall = ctx.enter_context(tc.tile_pool(name="usmall", bufs=6))
            ypool = ctx.enter_context(tc.tile_pool(name="yout", bufs=1))
            gpool = ctx.enter_context(tc.tile_pool(name="gpsum", bufs=7, space="PSUM"))
            fpool = ctx.enter_context(tc.tile_pool(name="fpsum", bufs=1, space="PSUM"))

            # --- inputs.  Consumption order: chunks 15..1 (need only the last
            # 1-3 X blocks + their WIHT pieces), then chunk 0 which scans
            # blocks 0..15 in ascending time order.  X arrival order matches:
            # 15,14,13 first, then 0,1,2,...,12, split across the two HWDGE
            # queues (sync + scalar). ---
            wiht_t = consts.tile([128, NCH * NI * 128], bf16, tag="wiht", name="wiht_t")
            scn_t = consts.tile([128, offs_total], bf16, tag="scn", name="scn_t")
            BW = NI * TB * BC  # 2048 cols per X block
            xt_tiles = [
                xpool.tile([128, BW], bf16, tag=f"x{kb}", name=f"x_{kb}")
                for kb in range(NBLK)
            ]
            xts = [t[:] for t in xt_tiles]
            # DMA split across both HWDGE queues (SP + ACT) so descriptor
            # generation runs in parallel; ACT's queue drains well before its
            # first COPY is ready to run.
            def wp(g0, ng):  # wiht piece slice
                return ds(g0 * NI * 128, ng * NI * 128)

            nc.sync.dma_start(wiht_t[:, wp(NCH - 1, 1)], WIHT[:, wp(NCH - 1, 1)])
            nc.sync.dma_start(xts[NBLK - 1], X[NBLK - 1])
            nc.sync.dma_start(wiht_t[:, wp(NCH - 3, 2)], WIHT[:, wp(NCH - 3, 2)])
            nc.sync.dma_start(xts[NBLK - 2], X[NBLK - 2])
            nc.sync.dma_start(xts[NBLK - 3], X[NBLK - 3])
            nc.sync.dma_start(wiht_t[:, wp(0, NCH - 3)], WIHT[:, wp(0, NCH - 3)])
            nc.sync.dma_start(scn_t[:], SCN)
            # chunk-0 blocks in ascending (scan) order; the last block split
            # by i-chunk so its GEMM pipelines with the arrival
            for kb in range(0, NBLK - 4):
                nc.sync.dma_start(xts[kb], X[kb])
            last = NBLK - 4
            for ic in range(NI):
                nc.sync.dma_start(
                    xts[last][:, ds(ic * TB * BC, TB * BC)],
                    X[last][:, ds(ic * TB * BC, TB * BC)],
                )
            bias_t = ypool.tile([BC, O], f32, tag="bias", name="bias_t")
            nc.sync.dma_start(bias_t[:], BIAS)
            whot_t = consts.tile([128, NCH * O], bf16, tag="whot", name="whot_t")
            nc.sync.dma_start(whot_t[:], WHOT)

            h_all = consts.tile([128, NCH * BC], bf16, tag="hall", name="h_all")

            # PE warm-up: dependency-free matmuls at t=0 lift the HAM clock
            # gate to 8/8 before the first real matmul arrives (~3.4us window)
            warm = consts.tile([128, TB * BC], f32, tag="warm", name="warm")
            nc.gpsimd.memset(warm[:], 0.0)
            wps = gpool.tile([128, TB * BC], f32, tag="gp", name="warm_ps")
            NWARM = 6
            for i in range(NWARM):
                nc.tensor.matmul(
                    wps[:],
                    warm[:, ds(0, 128)],
                    warm[:],
                    start=(i == 0),
                    stop=(i == NWARM - 1),
                )
            nc.scalar.copy(warm[:], wps[:])  # consume so the tiles are live

            # --- chunk-major pipeline: GEMM (PE) -> copy (ACT) -> scan (DVE) ---
            psy = fpool.tile([BC, O], f32, tag="fy", name="psy")
            proj_emitted = [0]

            def emit_proj(g):
                nc.tensor.matmul(
                    psy[:],
                    h_all[:, ds(g * BC, BC)],
                    whot_t[:, ds(g * O, O)],
                    start=(proj_emitted[0] == 0),
                    stop=(proj_emitted[0] == NCH - 1),
                )
                proj_emitted[0] += 1

            chunk_order = list(range(NCH - 1, 0, -1)) + [0]
            for g in chunk_order:
                kg = windows[g]
                st0 = S - kg  # first step of the window
                fb = st0 // TB  # first X block touched
                toff = st0 % TB  # in-block start column (0 or WG)
                if g == 0:
                    u_t = ubig.tile([128, BC * kg], f32, tag="u0", name="u_g0")
                    u3 = u_t[:].rearrange("p (b t) -> p b t", b=BC)
                else:
                    # per-batch width kg+2: the last two columns are the BIG
                    # separator pair that resets the fused scan between rows
                    u_t = usmall.tile(
                        [128, BC * (USMALL_W + 2)], f32, tag="us", name=f"u_g{g}"
                    )
                    u3 = u_t[:, ds(0, BC * (kg + 2))].rearrange(
                        "p (b t) -> p b t", b=BC
                    )
                    nc.gpsimd.memset(u3[:, :, ds(kg, 2)], 1.0e30)
                # chunk 0 consumes blocks in ascending (scan) order so each
                # GEMM run's segment scan chains off the previous one; other
                # chunks take newest-first (their X arrives first).
                blocks = (
                    list(range(fb, NBLK))
                    if g == 0
                    else list(range(NBLK - 1, fb - 1, -1))
                )
                def t0_of(kb):
                    return toff if kb == fb else 0

                def emit_gemm_copy(run):
                    ps = {
                        kb: gpool.tile(
                            [128, (TB - t0_of(kb)) * BC], f32, tag="gp", name=f"gp_{g}_{kb}"
                        )
                        for kb in run
                    }
                    for ic in range(NI):
                        for kb in run:
                            t0 = t0_of(kb)
                            rhs = xts[kb][:, ds(ic * TB * BC, TB * BC)]
                            out_ap = ps[kb][:]
                            if t0:
                                rhs = rhs.rearrange("p (b t) -> p b t", b=BC)[
                                    :, :, ds(t0, TB - t0)
                                ]
                                out_ap = out_ap.rearrange("p (b t) -> p b t", b=BC)
                            nc.tensor.matmul(
                                out_ap,
                                wiht_t[:, ds(g * NI * 128 + ic * 128, 128)],
                                rhs,
                                start=(ic == 0),
                                stop=(ic == NI - 1),
                            )
                    for kb in run:
                        t0 = t0_of(kb)
                        pos = kb * TB - st0 if kb > fb else 0
                        dst = u3[:, :, ds(pos, TB - t0)]
                        src = ps[kb][:].rearrange("p (b t) -> p b t", b=BC)
                        nc.scalar.copy(dst, src)

                def emit_seg_scans(first_blk, n_blk):
                    # segment scan seeded by the previous segment's last
                    # element per (lane, b); first segment seeds with zero
                    seg0 = (first_blk - fb) * TB
                    seg = n_blk * TB
                    scn_s = scn_t[:, ds(int(offs[g]) + seg0, seg)]
                    for b in range(BC):
                        ap = u_t[:, ds(b * kg + seg0, seg)]
                        if seg0 == 0:
                            nc.vector._custom_dve(SCAN_OP, out=ap, in0=ap, in1=scn_s)
                        else:
                            seed = u_t[:, ds(b * kg + seg0 - 1, 1)]
                            nc.vector._custom_dve(
                                SCAN_SEED_OP, out=ap, in0=ap, in1=scn_s, s0=seed
                            )

                if g == 0 and len(blocks) >= R + 6 and (len(blocks) - 4 - R) % 2 == 0:
                    # One R-run, then 2-block runs (so each pair's scan rides
                    # right behind its X arrival), with the last 4 blocks
                    # [w, x, y, z] GEMMed as [x, y, z] (X already resident)
                    # then [w] (the last DMA arrival) and scanned as a single
                    # 4-block segment: the post-DMA tail is G+C of w plus one
                    # segment scan.
                    n = len(blocks)
                    runs_scan = [blocks[:R]] + [
                        blocks[i : i + 2] for i in range(R, n - 4, 2)
                    ]
                    for run in runs_scan:
                        emit_gemm_copy(run)
                        emit_seg_scans(run[0], len(run))
                    emit_gemm_copy(blocks[n - 3 :])
                    emit_gemm_copy(blocks[n - 4 : n - 3])
                    emit_seg_scans(blocks[n - 4], 4)
                elif g == 0:
                    full, left = [], list(blocks)
                    while len(left) > 4:
                        full.append(left[:R])
                        left = left[R:]
                    for run in full:
                        emit_gemm_copy(run)
                        emit_seg_scans(run[0], len(run))
                    if len(left) > 1:
                        emit_gemm_copy(left[1:])
                        emit_gemm_copy(left[:1])
                        emit_seg_scans(left[0], 1)
                        emit_seg_scans(left[1], len(left) - 1)
                    else:
                        emit_gemm_copy(left)
                        emit_seg_scans(left[0], 1)
                else:
                    for rs in range(0, len(blocks), R):
                        emit_gemm_copy(blocks[rs : rs + R])
                if g != 0:
                    # one fused scan over all batch rows: the (-1, +1)-scaled
                    # BIG separator pair exactly zeroes the state between rows
                    scn_g = (
                        scn_t[:, ds(int(offs[g]), kg + 2)]
                        .rearrange("p (o t) -> p o t", o=1)
                        .broadcast_to([128, BC, kg + 2])
                    )
                    nc.vector._custom_dve(SCAN_OP, out=u3, in0=u3, in1=scn_g)
                # h_final = last scan element per (lane, b) -> bf16
                hsrc = u3[:, :, kg - 1]
                nc.vector.tensor_copy(h_all[:, ds(g * BC, BC)], hsrc)

            # --- final projection tail: any proj matmuls not yet emitted
            # (normally just chunk 0's, closing the PSUM accumulation) ---
            for g in chunk_order[proj_emitted[0] :]:
                emit_proj(g)
            y_t = ypool.tile([BC, O], f32, tag="y", name="y_t")
            nc.vector.tensor_tensor(y_t[:], psy[:], bias_t[:], mybir.AluOpType.add)
            nc.sync.dma_start(Y, y_t[:])
    nc.compile()
    return nc


def _get_program(windows, offs_total):
    key = (
        windows,
        os.environ.get("DIAG_R"),
        os.environ.get("DIAG_LN"),
    )
    if key not in _CACHE:
        _CACHE[key] = _build(windows, offs_total)
    return _CACHE[key]


def _ensure_ntff_hook():
    """Provide antenv.axon_hooks (absent in this image) so trace=True works."""
    import sys
    import types

    if "antenv.axon_hooks" in sys.modules:
        return True
    try:
        import antenv

        mod = types.ModuleType("antenv.axon_hooks")
        mod._hook = None

        def set_axon_ntff_profile_hook(h):
            mod._hook = h

        def get_axon_ntff_profile_hook():
            return mod._hook

        mod.set_axon_ntff_profile_hook = set_axon_ntff_profile_hook
        mod.get_axon_ntff_profile_hook = get_axon_ntff_profile_hook
        sys.modules["antenv.axon_hooks"] = mod
        antenv.axon_hooks = mod

        from trn_agent_boot.trn_boot import _ntff_profile_via_ctypes

        hook = _ntff_profile_via_ctypes("/opt/axon/libaxon_pjrt.so")
        mod.set_axon_ntff_profile_hook(hook)
        return hook is not None
    except Exception:
        return False


def _maybe_patch_ldw_opt():
    """Optionally flip walrus --enable-ldw-opt to true (DIAG_LDWOPT=1)."""
    from concourse import bass_utils

    if getattr(bass_utils, "_ant_ldw_patched", False):
        return
    if not bool(int(os.environ.get("DIAG_LDWOPT", "0"))):
        return
    orig = bass_utils.run_command

    def patched(argv, **kwargs):
        argv = [
            "--enable-ldw-opt=true" if a == "--enable-ldw-opt=false" else a
            for a in argv
        ]
        return orig(argv, **kwargs)

    bass_utils.run_command = patched
    bass_utils._ant_ldw_patched = True


def kernel(X, W_ih, hh, W_ho, b_ho):
    import ml_dtypes

    from concourse import bass_utils

    _maybe_patch_ldw_opt()

    X = np.asarray(X, dtype=np.float32)
    W_ih = np.asarray(W_ih, dtype=np.float32)
    hh = np.asarray(hh, dtype=np.float32)
    W_ho = np.asarray(W_ho, dtype=np.float32)
    b_ho = np.asarray(b_ho, dtype=np.float32)

    plan = _make_plan(hh)
    perm = plan["perm"]
    nc = _get_program(plan["windows"], int(plan["offs"][-1]))

    bf = ml_dtypes.bfloat16
    # WIHT [128, NCH*NI*128]: line p = [g, ic, hsub], W_ih[h=g*128+hsub, i=ic*128+p]
    wiht = np.ascontiguousarray(
        W_ih[perm].T.reshape(NI, 128, NCH, 128).transpose(1, 2, 0, 3).reshape(128, -1)
    ).astype(bf)
    # WHOT [128, NCH*O]: line p = [g, o] with value W_ho[o, h=g*128+p]
    whot = np.ascontiguousarray(
        W_ho[:, perm].T.reshape(NCH, 128, O).transpose(1, 0, 2).reshape(128, NCH * O)
    ).astype(bf)
    bias = np.tile(b_ho[None, :], (BC, 1)).astype(np.float32)

    common = {
        "WIHT": wiht,
        "WHOT": whot,
        "BIAS": bias,
        "SCN": plan["SCN"].astype(bf),
    }
    in_maps = []
    for m in range(NCORES):
        im = dict(common)
        xm = X[:, m * BC : (m + 1) * BC, :]  # [S, BC, I]
        # device layout [NBLK, 128(i-in-chunk), (ic, b, tau)]
        xt = xm.transpose(2, 1, 0).reshape(NI, 128, BC, NBLK, TB)
        xt = np.ascontiguousarray(xt.transpose(3, 1, 0, 2, 4)).reshape(
            NBLK, 128, NI * BC * TB
        )
        im["X"] = xt.astype(bf)
        in_maps.append(im)

    trace = bool(int(os.environ.get("DIAG_TRACE", "0")))
    if trace:
        trace = _ensure_ntff_hook()
    res = None
    for attempt in range(3):
        try:
            res = bass_utils.run_bass_kernel_spmd(
                nc,
                in_maps,
                core_ids=list(range(NCORES)),
                trace=trace,
                tmpdir=os.environ.get("DIAG_TRACE_DIR") or None,
            )
            break
        except Exception:
            if attempt == 2:
                raise
            trace = False  # retry without profiling
    if res.exec_time_ns is not None:
        kernel.last_exec_time_ns = res.exec_time_ns
        kernel.last_mean_exec_time_ns = res.mean_exec_time_ns
    Yfull = np.concatenate([r["Y"] for r in res.results], axis=0)
    return Yfull


kernel.last_exec_time_ns = None
kernel.last_mean_exec_time_ns = None



# revision 2
# speedup vs baseline: 1.1905x; 1.1905x over previous
"""Trainium2 Bass kernel for nn_Diagnet (S=1024, B=64, I=512, H=2048, O=512).

    u = einsum('sbi,hi->sbh', X, W_ih)
    h_t = |u_t + hh * h_{t-1}|   (scan over S, only final h needed)
    Y = h_final @ W_ho.T + b_ho

Strategy (8 NeuronCores, data-parallel over batch, BC=8 rows per core):

* H lanes are permuted so hh is sorted descending and split into 16
  chunks of 128.  A chunk whose largest decay a satisfies a^K < tol
  only needs the last K steps, so each chunk gets a window K_g and the
  GEMM + scan skip everything earlier.  Chunk 0 keeps the full 1024.
* The recurrence is a custom DVE instruction that folds a window in
  one go: m[t] = |m[t-1] - u[t]*scn[t]| with scn[t] = -a^(K-1-t)
  (prescale folds the decay into the stream; the minus sign turns
  ABSOLUTE_DIFF into abs-add).  h_final = last element.
* All 8 batch rows fold in ONE scan instruction per segment via a
  3-column header per row: a BIG separator pair scaled (-1, +1)
  absorbs and exactly zeroes the running state between rows, then a
  seed column (scn=-1) re-injects that row's carry from the previous
  segment (h >= 0 so |0 - s*(-1)| = s).  Seed values are copied
  between segments by a tiny DVE copy, keeping the serial chain on
  one engine.
* Chunk-0 blocks t<640 run in fp8 (e4m3) with DoubleRow perf mode
  (256-deep contraction, half the matmul passes and half the X bytes);
  late blocks and everything else stay bf16.  Decay weighting keeps
  the fp8 quantization error ~1.5% of max|Y| (gate 2e-2).
* One HWDGE queue streams, in priority order: W_ih tail chunks, X
  blocks 15/14/13 (all small chunks need only these), SCN, remaining
  W_ih, fp8 W/X blocks 0..9, W_ho (split hi/lo to feed interleaved
  output projections), bf16 X blocks 10..12.  The PE chases arrivals:
  small chunks first, then chunk-0 ascending; the 16 output-projection
  matmuls interleave into DMA slack instead of trailing at the end.
"""

import math
import os

from contextlib import ExitStack

import numpy as np

S, B, I, H, O = 1024, 64, 512, 2048, 512
NCORES = 8
BC = B // NCORES  # 8 batch rows per core
TB = 64  # X block granularity
WG = 32  # truncation-window granularity
NBLK = S // TB  # 16
NCH = H // 128  # 16 h-chunks
NI = I // 128  # 4 i-chunks
USMALL_W = 256  # max window (cols) for chunks g>=1
# chunk-0 scan segments as (first_block, n_blocks); full coverage of 0..15
SEGS = [(0, 2), (2, 2), (4, 2), (6, 2), (8, 2), (10, 2), (12, 4)]
HDR = 3  # per-row header cols: BIG sep (-1), BIG sep (+1), seed (-1)

_CACHE = {}


def _fp8_nblk():
    n = int(os.environ.get("DIAG_FP8BLK", "10"))
    assert n % 2 == 0 and 0 <= n <= (S - USMALL_W) // TB, n
    return n


def _seg_meta():
    """Per-segment (start_col, width) in per-row cols, and block->seg map."""
    seg_w = [HDR + nb * TB for _, nb in SEGS]
    seg_c = np.concatenate([[0], np.cumsum(seg_w)]).astype(int)
    seg_of = {}
    for s, (fb, nb) in enumerate(SEGS):
        for i in range(nb):
            seg_of[fb + i] = (s, i)
    return seg_w, seg_c, seg_of  # widths, col starts (len nseg+1), block map


def _register_scan_ops():
    """Fold op: m[t] = |m[t-1] - in0[t]*in1[t]|, zero-initialized."""
    import concourse.dve_ops as dve_ops
    from concourse.dve_spec import C0, Spec, Src0, Src1, Zero, scan, lower, AluOp
    from concourse.dve_uop import DveOpSpec

    have = {op.name: op for op in dve_ops.OPS}
    if "ABSDIFF_SCALE_SCAN_ANT" in have:
        return have["ABSDIFF_SCALE_SCAN_ANT"]

    def _ref(in0, in1, s0, s1, imm2):
        x = in0.astype(np.float32) * in1.astype(np.float32)
        out = np.empty_like(x)
        m = np.zeros(x.shape[0], np.float32)
        for t in range(x.shape[1]):
            m = np.abs(m - x[:, t])
            out[:, t] = m
        return out

    spec = Spec(
        body=scan(AluOp.ABSOLUTE_DIFF, Src0 * Src1, init=Zero),
        reference=_ref,
    )
    row = max(dve_ops._SUB_OPCODE_FOR_NAME.values()) + 1
    assert row < 0x20
    shas = {}
    for ver in ("v3", "v4"):
        s = DveOpSpec(
            name="ABSDIFF_SCALE_SCAN_ANT", opcode=row, uops=lower(spec, ver=ver),
            rd1_en=True,
        )
        shas[ver] = s.sha(ver)
    op = dve_ops.DveOp("ABSDIFF_SCALE_SCAN_ANT", spec, subdim=False, uops_sha=shas)
    dve_ops._SUB_OPCODE_FOR_NAME["ABSDIFF_SCALE_SCAN_ANT"] = row
    dve_ops.OPS.append(op)
    dve_ops.CUSTOM_DVE_SPECS["ABSDIFF_SCALE_SCAN_ANT"] = spec
    return op


def _windows(hh):
    ln = float(os.environ.get("DIAG_LN", "7.0"))
    a = np.maximum(np.abs(hh.astype(np.float64)), 1e-30)
    perm = np.argsort(-a, kind="stable")
    ag = a[perm].reshape(NCH, 128)
    windows = []
    for g in range(NCH):
        amax = ag[g, 0]
        if S * math.log(amax) >= -ln:
            kg = S
        else:
            kg = int(math.ceil(ln / math.log(1.0 / amax)))
        kg = min(S, max(WG, ((kg + WG - 1) // WG) * WG))
        windows.append(kg)
    assert windows[0] == S, windows
    assert all(windows[g] >= windows[g + 1] for g in range(NCH - 1)), windows
    assert all(k <= USMALL_W for k in windows[1:]), (windows, "raise USMALL_W")
    return perm, ag, tuple(windows)


def _small_offs(windows, base):
    """Start col of each small chunk's scn piece (g>=1), after chunk-0 base."""
    widths = [0] + [windows[g] + 2 for g in range(1, NCH)]
    return base + np.cumsum(widths).astype(int)  # index by g-1 ... use [g-1]


def _make_plan(hh):
    perm, ag, windows = _windows(hh)
    seg_w, seg_c, _ = _seg_meta()
    u0w = int(seg_c[-1])  # per-row cols of chunk-0 stream (1045)
    offs = np.concatenate(
        [[u0w], u0w + np.cumsum([windows[g] + 2 for g in range(1, NCH)])]
    ).astype(int)
    total = int(offs[-1])
    scn = np.zeros((128, total), dtype=np.float64)
    a0 = ag[0]
    col = 0
    for (fb, nb) in SEGS:
        scn[:, col] = -1.0
        scn[:, col + 1] = 1.0
        scn[:, col + 2] = -1.0
        t = np.arange(fb * TB, (fb + nb) * TB)
        scn[:, col + HDR : col + HDR + nb * TB] = -(
            a0[:, None] ** (S - 1 - t)[None, :]
        )
        col += HDR + nb * TB
    assert col == u0w
    for g in range(1, NCH):
        kg = windows[g]
        off = int(offs[g - 1])
        tau = np.arange(kg)
        scn[:, off : off + kg] = -(ag[g][:, None] ** (kg - 1 - tau)[None, :])
        scn[:, off + kg] = -1.0
        scn[:, off + kg + 1] = 1.0
    return {"perm": perm, "windows": windows, "offs": offs, "SCN": scn}


def _build(windows):
    import concourse.mybir as mybir
    import concourse.tile as tile
    from concourse import bacc
    from concourse.bass import ds

    SCAN_OP = _register_scan_ops()
    f32 = mybir.dt.float32
    bf16 = mybir.dt.bfloat16
    f8 = mybir.dt.float8e4
    DR = mybir.MatmulPerfMode.DoubleRow
    NF8 = _fp8_nblk()

    seg_w, seg_c, seg_of = _seg_meta()
    u0w = int(seg_c[-1])
    offs = np.concatenate(
        [[u0w], u0w + np.cumsum([windows[g] + 2 for g in range(1, NCH)])]
    ).astype(int)
    total_scn = int(offs[-1])
    nseg = len(SEGS)

    nc = bacc.Bacc("TRN2", target_bir_lowering=False, debug=False, num_devices=NCORES)
    XB = nc.dram_tensor(
        "XB", [NBLK - NF8, 128, NI * TB * BC], bf16, kind="ExternalInput"
    ).ap()  # blocks NF8..15, line [ic, b, tau]
    if NF8:
        X8 = nc.dram_tensor(
            "X8", [NF8, 128, 4 * TB * BC], f8, kind="ExternalInput"
        ).ap()  # blocks 0..NF8-1, line [j, k, b, tau], i = (2j+k)*128+p
        W8 = nc.dram_tensor("W8", [128, 4 * 128], f8, kind="ExternalInput").ap()
    WIHT = nc.dram_tensor("WIHT", [128, NCH * NI * 128], bf16, kind="ExternalInput").ap()
    WHOT = nc.dram_tensor("WHOT", [128, NCH * O], bf16, kind="ExternalInput").ap()
    SCN = nc.dram_tensor("SCN", [128, total_scn], bf16, kind="ExternalInput").ap()
    BIAS = nc.dram_tensor("BIAS", [BC, O], f32, kind="ExternalInput").ap()
    Y = nc.dram_tensor("Y", [BC, O], f32, kind="ExternalOutput").ap()

    with tile.TileContext(nc) as tc:
        with ExitStack() as ctx:
            consts = ctx.enter_context(tc.tile_pool(name="consts", bufs=1))
            xpool = ctx.enter_context(tc.tile_pool(name="xt", bufs=1))
            ubig = ctx.enter_context(tc.tile_pool(name="ubig", bufs=1))
            usmall = ctx.enter_context(tc.tile_pool(name="usmall", bufs=4))
            ypool = ctx.enter_context(tc.tile_pool(name="yout", bufs=1))
            gpool = ctx.enter_context(tc.tile_pool(name="gpsum", bufs=7, space="PSUM"))
            fpool = ctx.enter_context(tc.tile_pool(name="fpsum", bufs=1, space="PSUM"))

            wiht_t = consts.tile([128, NCH * NI * 128], bf16, tag="wiht", name="wiht_t")
            whot_t = consts.tile([128, NCH * O], bf16, tag="whot", name="whot_t")
            scn_t = consts.tile([128, total_scn], bf16, tag="scn", name="scn_t")
            bias_t = ypool.tile([BC, O], f32, tag="bias", name="bias_t")
            h_all = consts.tile([128, NCH * BC], bf16, tag="hall", name="h_all")
            if NF8:
                w8_t = consts.tile([128, 4 * 128], f8, tag="w8", name="w8_t")
            x8ts = [
                xpool.tile([128, 4 * TB * BC], f8, tag=f"x8_{kb}", name=f"x8_{kb}")[:]
                for kb in range(NF8)
            ]
            xbts = {
                kb: xpool.tile(
                    [128, NI * TB * BC], bf16, tag=f"xb_{kb}", name=f"xb_{kb}"
                )[:]
                for kb in range(NF8, NBLK)
            }
            u0_t = ubig.tile([128, BC * u0w], f32, tag="u0", name="u0")

            def u0_seg(s):  # [p, b, width_s]
                return u0_t[:, ds(int(seg_c[s]) * BC, BC * seg_w[s])].rearrange(
                    "p (b t) -> p b t", b=BC
                )

            # --- DMA stream (single HWDGE queue; order = priority) ---
            def wp(g0, ng):
                return ds(g0 * NI * 128, ng * NI * 128)

            dma = nc.sync.dma_start
            dma(wiht_t[:, wp(13, 3)], WIHT[:, wp(13, 3)])
            dma(xbts[15], XB[15 - NF8])
            dma(scn_t[:], SCN)
            dma(xbts[14], XB[14 - NF8])
            dma(xbts[13], XB[13 - NF8])
            dma(wiht_t[:, wp(10, 3)], WIHT[:, wp(10, 3)])
            dma(wiht_t[:, wp(7, 3)], WIHT[:, wp(7, 3)])
            dma(wiht_t[:, wp(4, 3)], WIHT[:, wp(4, 3)])
            dma(wiht_t[:, wp(0, 4)], WIHT[:, wp(0, 4)])
            if NF8:
                dma(w8_t[:], W8)
            dma(whot_t[:, ds(8 * O, 8 * O)], WHOT[:, ds(8 * O, 8 * O)])  # g8..15
            for kb in range(NF8):
                dma(x8ts[kb], X8[kb])
            dma(whot_t[:, ds(0, 8 * O)], WHOT[:, ds(0, 8 * O)])  # g0..7
            for kb in range(NF8, 13):
                dma(xbts[kb], XB[kb - NF8])
            dma(bias_t[:], BIAS)

            # --- header memsets for the fused scans ---
            for s in range(nseg):
                nc.gpsimd.memset(u0_seg(s)[:, :, ds(0, 2)], 1.0e30)
            nc.gpsimd.memset(u0_seg(0)[:, :, ds(2, 1)], 0.0)

            # --- PE warm-up (lifts the HAM clock gate before real work) ---
            warm = consts.tile([128, TB * BC], f32, tag="warm", name="warm")
            nc.gpsimd.memset(warm[:], 0.0)
            wps = gpool.tile([128, TB * BC], f32, tag="gp", name="warm_ps")
            NWARM = 6
            for i in range(NWARM):
                nc.tensor.matmul(
                    wps[:], warm[:, ds(0, 128)], warm[:],
                    start=(i == 0), stop=(i == NWARM - 1),
                )
            nc.scalar.copy(warm[:], wps[:])

            # --- output projection bookkeeping ---
            psy = fpool.tile([BC, O], f32, tag="fy", name="psy")
            proj_pending = list(range(NCH - 1, 0, -1))  # g15..g1; g0 last
            proj_done = [0]

            def emit_proj_g(g):
                nc.tensor.matmul(
                    psy[:], h_all[:, ds(g * BC, BC)], whot_t[:, ds(g * O, O)],
                    start=(proj_done[0] == 0), stop=(proj_done[0] == NCH - 1),
                )
                proj_done[0] += 1

            def emit_projs(n):
                for _ in range(min(n, len(proj_pending))):
                    emit_proj_g(proj_pending.pop(0))

            # --- small chunks g15..g1 (X blocks 13..15, resident early) ---
            for g in range(NCH - 1, 0, -1):
                kg = windows[g]
                st0 = S - kg
                fb = st0 // TB
                toff = st0 % TB
                u_t = usmall.tile(
                    [128, BC * (USMALL_W + 2)], f32, tag="us", name=f"u_g{g}"
                )
                u3 = u_t[:, ds(0, BC * (kg + 2))].rearrange("p (b t) -> p b t", b=BC)
                nc.gpsimd.memset(u3[:, :, ds(kg, 2)], 1.0e30)
                blocks = list(range(fb, NBLK))

                def t0_of(kb, fb=fb, toff=toff):
                    return toff if kb == fb else 0

                ps = {
                    kb: gpool.tile(
                        [128, (TB - t0_of(kb)) * BC], f32, tag="gp", name=f"gp_{g}_{kb}"
                    )
                    for kb in blocks
                }
                for ic in range(NI):
                    for kb in blocks:
                        t0 = t0_of(kb)
                        rhs = xbts[kb][:, ds(ic * TB * BC, TB * BC)]
                        out_ap = ps[kb][:]
                        if t0:
                            rhs = rhs.rearrange("p (b t) -> p b t", b=BC)[
                                :, :, ds(t0, TB - t0)
                            ]
                            out_ap = out_ap.rearrange("p (b t) -> p b t", b=BC)
                        nc.tensor.matmul(
                            out_ap,
                            wiht_t[:, ds(g * NI * 128 + ic * 128, 128)],
                            rhs,
                            start=(ic == 0),
                            stop=(ic == NI - 1),
                        )
                for kb in blocks:
                    t0 = t0_of(kb)
                    pos = kb * TB - st0 if kb > fb else 0
                    nc.scalar.copy(
                        u3[:, :, ds(pos, TB - t0)],
                        ps[kb][:].rearrange("p (b t) -> p b t", b=BC),
                    )
                scn_g = (
                    scn_t[:, ds(int(offs[g - 1]), kg + 2)]
                    .rearrange("p (o t) -> p o t", o=1)
                    .broadcast_to([128, BC, kg + 2])
                )
                nc.vector._custom_dve(SCAN_OP, out=u3, in0=u3, in1=scn_g)
                nc.vector.tensor_copy(h_all[:, ds(g * BC, BC)], u3[:, :, kg - 1])

            # --- chunk 0: GEMM chases the DMA stream; fused per-seg scans ---
            seg_left = {s: nb for s, (_, nb) in enumerate(SEGS)}
            next_scan = [0]

            def copy_c0(kb, ps_t):
                s, i = seg_of[kb]
                nc.scalar.copy(
                    u0_seg(s)[:, :, ds(HDR + i * TB, TB)],
                    ps_t[:].rearrange("p (b t) -> p b t", b=BC),
                )
                seg_left[s] -= 1

            def emit_ready_scans():
                while next_scan[0] < nseg and seg_left[next_scan[0]] == 0:
                    s = next_scan[0]
                    w = seg_w[s]
                    if s > 0:
                        nc.vector.tensor_copy(
                            u0_seg(s)[:, :, 2], u0_seg(s - 1)[:, :, seg_w[s - 1] - 1]
                        )
                    scn_s = (
                        scn_t[:, ds(int(seg_c[s]), w)]
                        .rearrange("p (o t) -> p o t", o=1)
                        .broadcast_to([128, BC, w])
                    )
                    u3s = u0_seg(s)
                    nc.vector._custom_dve(SCAN_OP, out=u3s, in0=u3s, in1=scn_s)
                    next_scan[0] += 1

            def emit_c0_bf16_run(blocks):
                pss = {
                    kb: gpool.tile([128, TB * BC], f32, tag="gp", name=f"c0_{kb}")
                    for kb in blocks
                }
                for ic in range(NI):
                    for kb in blocks:
                        nc.tensor.matmul(
                            pss[kb][:],
                            wiht_t[:, ds(ic * 128, 128)],
                            xbts[kb][:, ds(ic * TB * BC, TB * BC)],
                            start=(ic == 0),
                            stop=(ic == NI - 1),
                        )
                for kb in blocks:
                    copy_c0(kb, pss[kb])

            def emit_c0_fp8_pair(k0):
                pair = (k0, k0 + 1)
                pss = {
                    kb: gpool.tile([128, TB * BC], f32, tag="gp", name=f"c8_{kb}")
                    for kb in pair
                }
                for j in range(2):
                    w_ap = w8_t[:].rearrange("p (j k h) -> p j k h", j=2, k=2)[:, j]
                    for kb in pair:
                        rhs = x8ts[kb].rearrange("p (j k n) -> p j k n", j=2, k=2)[
                            :, j
                        ]
                        nc.tensor.matmul(
                            pss[kb][:], w_ap, rhs,
                            start=(j == 0), stop=(j == 1), perf_mode=DR,
                        )
                for kb in pair:
                    copy_c0(kb, pss[kb])

            # blocks 13..15 are resident from the start: front-load their GEMM
            emit_c0_bf16_run([13, 14, 15])
            # fp8 pairs chase X8 arrivals; interleave g15..g8 projections
            for k0 in range(0, NF8, 2):
                emit_c0_fp8_pair(k0)
                emit_ready_scans()
                if k0 < 8:
                    emit_projs(2)
            # bf16 mid blocks (NF8..11) as pairs
            mids = list(range(NF8, 12))
            for k0 in mids[::2]:
                run = [kb for kb in (k0, k0 + 1) if kb < 12]
                emit_c0_bf16_run(run)
                emit_ready_scans()
                emit_projs(3)
            # final block 12 closes the last segment
            emit_c0_bf16_run([12])
            emit_ready_scans()
            assert next_scan[0] == nseg and not any(seg_left.values())
            emit_projs(len(proj_pending))
            # chunk-0 h extract, then its projection closes the accumulation
            nc.vector.tensor_copy(
                h_all[:, ds(0, BC)], u0_seg(nseg - 1)[:, :, seg_w[nseg - 1] - 1]
            )
            emit_proj_g(0)
            assert proj_done[0] == NCH

            y_t = ypool.tile([BC, O], f32, tag="y", name="y_t")
            nc.vector.tensor_tensor(y_t[:], psy[:], bias_t[:], mybir.AluOpType.add)
            nc.sync.dma_start(Y, y_t[:])
    nc.compile()
    return nc


def _get_program(windows):
    key = (windows, os.environ.get("DIAG_LN"), os.environ.get("DIAG_FP8BLK"))
    if key not in _CACHE:
        _CACHE[key] = _build(windows)
    return _CACHE[key]


def _ensure_ntff_hook():
    """Provide antenv.axon_hooks (absent in this image) so trace=True works."""
    import sys
    import types

    if "antenv.axon_hooks" in sys.modules:
        return True
    try:
        import antenv

        mod = types.ModuleType("antenv.axon_hooks")
        mod._hook = None

        def set_axon_ntff_profile_hook(h):
            mod._hook = h

        def get_axon_ntff_profile_hook():
            return mod._hook

        mod.set_axon_ntff_profile_hook = set_axon_ntff_profile_hook
        mod.get_axon_ntff_profile_hook = get_axon_ntff_profile_hook
        sys.modules["antenv.axon_hooks"] = mod
        antenv.axon_hooks = mod

        from trn_agent_boot.trn_boot import _ntff_profile_via_ctypes

        hook = _ntff_profile_via_ctypes("/opt/axon/libaxon_pjrt.so")
        mod.set_axon_ntff_profile_hook(hook)
        return hook is not None
    except Exception:
        return False


def kernel(X, W_ih, hh, W_ho, b_ho):
    import ml_dtypes

    from concourse import bass_utils

    X = np.asarray(X, dtype=np.float32)
    W_ih = np.asarray(W_ih, dtype=np.float32)
    hh = np.asarray(hh, dtype=np.float32)
    W_ho = np.asarray(W_ho, dtype=np.float32)
    b_ho = np.asarray(b_ho, dtype=np.float32)

    plan = _make_plan(hh)
    perm = plan["perm"]
    nc = _get_program(plan["windows"])
    NF8 = _fp8_nblk()

    bf = ml_dtypes.bfloat16
    f8 = ml_dtypes.float8_e4m3
    # WIHT [128, NCH*NI*128]: line p = [g, ic, hsub], W_ih[h=g*128+hsub, i=ic*128+p]
    wiht = np.ascontiguousarray(
        W_ih[perm].T.reshape(NI, 128, NCH, 128).transpose(1, 2, 0, 3).reshape(128, -1)
    ).astype(bf)
    # WHOT [128, NCH*O]: line p = [g, o] with value W_ho[o, h=g*128+p]
    whot = np.ascontiguousarray(
        W_ho[:, perm].T.reshape(NCH, 128, O).transpose(1, 0, 2).reshape(128, NCH * O)
    ).astype(bf)
    bias = np.tile(b_ho[None, :], (BC, 1)).astype(np.float32)

    common = {
        "WIHT": wiht,
        "WHOT": whot,
        "BIAS": bias,
        "SCN": plan["SCN"].astype(bf),
    }
    if NF8:
        # W8 [128, (j,k,h)]: chunk-0 weights, i = (2j+k)*128 + p
        w0 = W_ih[perm[:128]]  # [128h, I]
        common["W8"] = np.ascontiguousarray(
            w0.T.reshape(2, 2, 128, 128).transpose(2, 0, 1, 3).reshape(128, 512)
        ).astype(f8)
    in_maps = []
    for m in range(NCORES):
        im = dict(common)
        xm = X[:, m * BC : (m + 1) * BC, :]  # [S, BC, I]
        # bf16 blocks: [128(i%128), (ic, b, tau)]
        xt = xm.transpose(2, 1, 0).reshape(NI, 128, BC, NBLK, TB)
        xt = np.ascontiguousarray(xt.transpose(3, 1, 0, 2, 4)).reshape(
            NBLK, 128, NI * BC * TB
        )
        im["XB"] = xt[NF8:].astype(bf)
        if NF8:
            x8 = np.empty((NF8, 128, 4 * BC * TB), dtype=f8)
            for kb in range(NF8):
                blk = xm[kb * TB : (kb + 1) * TB]  # [TB, BC, I]
                a = blk.transpose(2, 1, 0).reshape(2, 2, 128, BC, TB)  # [j,k,p,b,t]
                x8[kb] = (
                    np.ascontiguousarray(a.transpose(2, 0, 1, 3, 4))
                    .reshape(128, 4 * BC * TB)
                    .astype(f8)
                )
            im["X8"] = x8
        in_maps.append(im)

    trace = bool(int(os.environ.get("DIAG_TRACE", "0")))
    if trace:
        trace = _ensure_ntff_hook()
    res = None
    for attempt in range(3):
        try:
            res = bass_utils.run_bass_kernel_spmd(
                nc,
                in_maps,
                core_ids=list(range(NCORES)),
                trace=trace,
                tmpdir=os.environ.get("DIAG_TRACE_DIR") or None,
            )
            break
        except Exception:
            if attempt == 2:
                raise
            trace = False  # retry without profiling
    if res.exec_time_ns is not None:
        kernel.last_exec_time_ns = res.exec_time_ns
        kernel.last_mean_exec_time_ns = res.mean_exec_time_ns
    Yfull = np.concatenate([r["Y"] for r in res.results], axis=0)
    return Yfull


kernel.last_exec_time_ns = None
kernel.last_mean_exec_time_ns = None
